# revision 1
# baseline (speedup 1.0000x reference)
"""Trainium2 Bass kernel for nn_MoELayerStacks (moe_routing).

Full inputs in, full output out. Data-parallel over batch across 8 cores.

Math (per batch row b):
  gate = [x[:32], x[1536:1568]] @ router_w.T + router_b           # [8]
  idx  = argmax(gate)
  l1c  = x @ l1_w[e].T + l1_b[e]   for all e                      # [8, 16]
  l1x  = clip([square(l1c[:, :15])*255/256, l1c[:, :15]], 0, 1)   # [8, 30]
  l2x  = clip(l1x @ l2_w[e].T + l2_b[e], 0, 1)                    # [8, 32]
  out  = (l2x @ out_w[e].T + out_b[e] + l1c[:, 15])[idx]          # [1]

Layout: features on partitions ("transposed"), batch on the free dim.
Stacked l1 feature index: f = e for o=15 (the l1x_out features, partitions
0..7 — compute engines can only start APs at partition 0/32/64/96), and
f = 8 + o*8 + e for o in 0..14.
l2 outputs are split into two expert groups (0-3, 4-7) of 128 features.
The final argmax-gather is done in batch-on-partitions layout after small
PE transposes of the [8, 512] gate / all-expert-output tiles.
"""

import os
from contextlib import ExitStack

import numpy as np

import concourse.bacc as bacc
import concourse.mybir as mybir
import concourse.tile as tile

N_CORES = 8
B, L1, L2, L3, E = 16384, 3072, 15, 32, 8
RF = 32  # router feats per perspective
HALF = L1 // 2
B_SH = B // N_CORES  # 2048 rows per core
KC = L1 // 128  # 24 contraction chunks
SQ_SCALE = float(np.sqrt(255.0 / 256.0))

F32 = mybir.dt.float32
ALU = mybir.AluOpType


def build_nc(b_sh=B_SH, mb=512, fast_mm=True):
    """Build the per-core Bass program. b_sh rows per core, mb rows per block.

    fast_mm: run the l1/l2/l3 matmuls in float32r (full-rate fp32 PE mode,
    1 cyc/row at N>=256 vs 4 for plain fp32). The router matmul stays plain
    fp32 — argmax selection needs the extra precision there.
    """
    nb = b_sh // mb
    assert mb % 128 == 0 and mb <= 512
    nsub = mb // 128  # sub-blocks of 128 for the transposes
    nc = bacc.Bacc(dynamic_dma_scratch_size=2048)

    mdt = mybir.dt.float32r if fast_mm else F32

    xT = nc.dram_tensor("xT", [L1, b_sh], mdt, kind="ExternalInput")
    w1t = nc.dram_tensor("w1t", [128, KC * 128], mdt, kind="ExternalInput")
    w2p = nc.dram_tensor("w2p", [128, 512], mdt, kind="ExternalInput")
    w3p = nc.dram_tensor("w3p", [128, 16], mdt, kind="ExternalInput")
    wr = nc.dram_tensor("wr", [2 * RF, E], F32, kind="ExternalInput")
    biasp = nc.dram_tensor("biasp", [128, 8], F32, kind="ExternalInput")
    idn = nc.dram_tensor("idn", [E, E], F32, kind="ExternalInput")
    y = nc.dram_tensor("y", [128, b_sh // 128], F32, kind="ExternalOutput")

    with tile.TileContext(nc) as tc, ExitStack() as ctx:
        const = ctx.enter_context(tc.tile_pool(name="const", bufs=1))
        xpool = ctx.enter_context(tc.tile_pool(name="x", bufs=11))
        xrpool = ctx.enter_context(tc.tile_pool(name="xr", bufs=3))
        actp = ctx.enter_context(tc.tile_pool(name="act", bufs=2))
        smallp = ctx.enter_context(tc.tile_pool(name="small", bufs=2))
        ps_big = ctx.enter_context(tc.tile_pool(name="ps1", bufs=2, space="PSUM"))
        ps_gate = ctx.enter_context(tc.tile_pool(name="psg", bufs=2, space="PSUM"))
        ps_2a = ctx.enter_context(tc.tile_pool(name="ps2a", bufs=1, space="PSUM"))
        ps_2b = ctx.enter_context(tc.tile_pool(name="ps2b", bufs=1, space="PSUM"))
        ps_3 = ctx.enter_context(tc.tile_pool(name="ps3", bufs=1, space="PSUM"))
        ps_t = ctx.enter_context(tc.tile_pool(name="pst", bufs=1, space="PSUM"))

        # --- constants; w1t pieces interleave with block-0 subtiles so the
        # first matmuls start as soon as ~1 MB has landed ---
        w1t_sb = const.tile([128, KC, 128], mdt)
        w1t_v = w1t[:, :].rearrange("p (c f) -> p c f", f=128)

        def load_w1t(eng, c0, n):
            eng.dma_start(w1t_sb[:, c0 : c0 + n, :], w1t_v[:, c0 : c0 + n, :])

        w2_sb = const.tile([128, 512], mdt)
        w3_sb = const.tile([128, 16], mdt)
        wr_sb = const.tile([2 * RF, E], F32)
        bias_sb = const.tile([128, 8], F32)
        idn_sb = const.tile([E, E], F32)
        yfull = const.tile([128, nb * nsub], F32)

        def emit_small_consts():
            nc.sync.dma_start(w2_sb[:], w2p[:, :])
            nc.sync.dma_start(w3_sb[:], w3p[:, :])
            nc.sync.dma_start(wr_sb[:], wr[:, :])
            nc.sync.dma_start(bias_sb[:], biasp[:, :])
            nc.sync.dma_start(idn_sb[:], idn[:, :])

        st = {}  # per-block live tiles for the skewed pipeline

        def load_piece(m0, c0, n, eng):
            xt = xpool.tile([128, 6, mb], mdt, tag="xt")
            eng.dma_start(
                xt[:, 0:n, :],
                xT[c0 * 128 : (c0 + n) * 128, m0 : m0 + mb].rearrange(
                    "(c p) m -> p c m", p=128
                ),
            )
            return xt

        def emit_load(b, first=False, last=False):
            m0 = b * mb
            cmap = [None] * KC

            def add(c0, n, eng):
                xt = load_piece(m0, c0, n, eng)
                for k in range(n):
                    cmap[c0 + k] = xt[:, k, :]

            if last:
                # fine trailing pieces: the final matmuls gate on less data
                add(0, 6, nc.sync)
                add(6, 6, nc.scalar)
                add(12, 6, nc.sync)
                add(18, 3, nc.scalar)
                add(21, 3, nc.sync)
            elif first:
                # fine-grained start: chunk 0 data lands after ~1 MB
                load_w1t(nc.sync, 0, 6)
                add(0, 3, nc.sync)
                add(3, 3, nc.scalar)
                load_w1t(nc.scalar, 6, 6)
                add(6, 6, nc.scalar)
                add(12, 6, nc.sync)
                load_w1t(nc.sync, 12, 6)
                load_w1t(nc.scalar, 18, 6)
                add(18, 6, nc.scalar)
            else:
                add(0, 6, nc.sync)
                add(6, 6, nc.scalar)
                add(12, 6, nc.sync)
                add(18, 6, nc.scalar)
            xr = xrpool.tile([2 * RF, mb], F32, tag="xr")
            nc.scalar.dma_start(xr[0:RF, :], xT[0:RF, m0 : m0 + mb].bitcast(F32))
            nc.scalar.dma_start(
                xr[RF : 2 * RF, :], xT[HALF : HALF + RF, m0 : m0 + mb].bitcast(F32)
            )
            st[b] = {"cmap": cmap, "xr": xr}

        def emit_burst(b):
            # l1: 24 accumulating matmuls -> l1c.T in PSUM [128f, mb]
            cmap = st[b]["cmap"]
            ps1 = ps_big.tile([128, mb], F32, tag="ps1")
            for c in range(KC):
                nc.tensor.matmul(
                    ps1[:],
                    w1t_sb[:, c, :],
                    cmap[c],
                    start=(c == 0),
                    stop=(c == KC - 1),
                )
            # router: gate.T [8, mb] (plain fp32 for precision)
            gps = ps_gate.tile([E, mb], F32, tag="gate")
            nc.tensor.matmul(gps[:], wr_sb[:], st[b]["xr"][:], start=True, stop=True)
            st[b]["ps1"] = ps1
            st[b]["gps"] = gps

        def emit_tail(b):
            m0 = b * mb
            ps1 = st[b]["ps1"]
            # all elementwise work on DVE so the Scalar engine queue stays a
            # pure DMA-issue stream (ACT ops would head-of-line block it)
            # sq = min(1, (s*(l1c+b1))^2)   (>=0 already)
            sq = actp.tile([128, mb], mdt, tag="sq")
            nc.vector.tensor_scalar(
                sq[:], ps1[:], bias_sb[:, 1:2], SQ_SCALE, op0=ALU.add, op1=ALU.mult
            )
            nc.vector.tensor_tensor(sq[:], sq[:], sq[:], op=ALU.mult)
            nc.vector.tensor_scalar_min(sq[:], sq[:], 1.0)
            # raw = min(1, max(0, l1c + b1))
            raw = actp.tile([128, mb], mdt, tag="raw")
            nc.vector.tensor_scalar(
                raw[:], ps1[:], bias_sb[:, 1:2], 0.0, op0=ALU.add, op1=ALU.max
            )
            nc.vector.tensor_scalar_min(raw[:], raw[:], 1.0)

            # l2: two expert groups, each sq+raw accumulated
            ps2a = ps_2a.tile([128, mb], F32, tag="ps2a")
            nc.tensor.matmul(ps2a[:], w2_sb[:, 0:128], sq[:], start=True, stop=False)
            nc.tensor.matmul(ps2a[:], w2_sb[:, 128:256], raw[:], start=False, stop=True)
            ps2b = ps_2b.tile([128, mb], F32, tag="ps2b")
            nc.tensor.matmul(ps2b[:], w2_sb[:, 256:384], sq[:], start=True, stop=False)
            nc.tensor.matmul(ps2b[:], w2_sb[:, 384:512], raw[:], start=False, stop=True)

            l2a = actp.tile([128, mb], mdt, tag="l2a")
            nc.vector.tensor_scalar(
                l2a[:], ps2a[:], bias_sb[:, 2:3], 0.0, op0=ALU.add, op1=ALU.max
            )
            nc.vector.tensor_scalar_min(l2a[:], l2a[:], 1.0)
            l2b = actp.tile([128, mb], mdt, tag="l2b")
            nc.vector.tensor_scalar(
                l2b[:], ps2b[:], bias_sb[:, 3:4], 0.0, op0=ALU.add, op1=ALU.max
            )
            nc.vector.tensor_scalar_min(l2b[:], l2b[:], 1.0)

            # l3: both groups accumulate into [8, mb]
            ps3 = ps_3.tile([E, mb], F32, tag="ps3")
            nc.tensor.matmul(ps3[:], w3_sb[:, 0:8], l2a[:], start=True, stop=False)
            nc.tensor.matmul(ps3[:], w3_sb[:, 8:16], l2b[:], start=False, stop=True)

            # all_outputs.T = l3c + l1x_out + (b1[:,15] + out_b)
            lout = smallp.tile([E, mb], F32, tag="lout")
            nc.vector.tensor_scalar(
                lout[:], ps1[0:8, :], bias_sb[0:8, 4:5], None, op0=ALU.add
            )
            all_sb = smallp.tile([E, mb], F32, tag="all")
            nc.vector.tensor_tensor(all_sb[:], ps3[:], lout[:], op=ALU.add)
            # gate + router_b  (also moves PSUM->SBUF for the transpose)
            gate_sb = smallp.tile([E, mb], F32, tag="gatesb")
            nc.vector.tensor_scalar(
                gate_sb[:], st[b]["gps"][:], bias_sb[0:8, 5:6], None, op0=ALU.add
            )

            # transpose to batch-on-partitions; gbt/abt share one PSUM bank
            tb = ps_t.tile([128, 2, nsub, E], F32, tag="tb")
            gbt, abt = tb[:, 0], tb[:, 1]
            for j in range(nsub):
                nc.tensor.transpose(
                    gbt[:, j, :], gate_sb[:, j * 128 : (j + 1) * 128], idn_sb[:]
                )
                nc.tensor.transpose(
                    abt[:, j, :], all_sb[:, j * 128 : (j + 1) * 128], idn_sb[:]
                )

            # argmax-select
            mx = smallp.tile([128, nsub], F32, tag="mx")
            nc.vector.reduce_max(mx[:], gbt[:], axis=mybir.AxisListType.X)
            eq = smallp.tile([128, nsub, E], F32, tag="eq")
            for j in range(nsub):
                nc.vector.tensor_scalar(
                    eq[:, j, :], gbt[:, j, :], mx[:, j : j + 1], None, op0=ALU.is_ge
                )
            prod = smallp.tile([128, nsub, E], F32, tag="prod")
            nc.vector.tensor_tensor(prod[:], eq[:], abt[:], op=ALU.mult)
            nc.vector.reduce_sum(
                yfull[:, b * nsub : (b + 1) * nsub], prod[:], axis=mybir.AxisListType.X
            )
            del st[b]

        # skewed software pipeline: PE does burst(b) before tail(b-1), so the
        # next block's dense matmul work is never queued behind the previous
        # block's latency-bound tail chain.
        emit_load(0, first=True)
        emit_small_consts()
        emit_burst(0)
        for b in range(1, nb):
            emit_load(b, last=(b == nb - 1))
            emit_burst(b)
            emit_tail(b - 1)
        emit_tail(nb - 1)
        # single contiguous output store; host un-permutes [p, k] -> row k*128+p
        nc.sync.dma_start(y[:, :], yfull[:])

    nc.finalize()
    return nc


def prep_weights(router_w, router_b, l1_w, l1_b, l2_w, l2_b, out_w, out_b):
    """Host-side packing of the (tiny) weights into the kernel's layouts."""
    f4 = np.float32
    # W1 stacked: row f = e for o=15 (l1x_out), f = 8 + o*8 + e for o < 15
    w1_stacked = np.concatenate(
        [l1_w[:, L2, :], np.transpose(l1_w[:, :L2, :], (1, 0, 2)).reshape(120, L1)],
        axis=0,
    )  # [128, L1]
    w1t_kf = np.ascontiguousarray(w1_stacked.T).astype(f4)  # [L1, 128]
    # swizzle to [p, c, f] so the on-chip load is one fully contiguous DMA
    w1t = np.ascontiguousarray(
        np.transpose(w1t_kf.reshape(KC, 128, 128), (1, 0, 2))
    ).reshape(128, KC * 128)
    # l2 block weights: rows f_in = 8+o*8+e, packed [sqA | rawA | sqB | rawB]
    w2p = np.zeros((128, 512), f4)
    for e in range(E):
        base = 0 if e < 4 else 256
        c0 = (e % 4) * 32
        wt = l2_w[e].T  # [30, 32]; rows 0..14 sq features, 15..29 raw
        rows = 8 + np.arange(L2) * 8 + e  # f for o in 0..14
        w2p[rows, base + c0 : base + c0 + 32] = wt[0:L2]
        w2p[rows, base + 128 + c0 : base + 128 + c0 + 32] = wt[L2 : 2 * L2]
    # l3: [128, 16] = [W3A | W3B], each block [128, 8] with out partition = e.
    # Block A covers experts 0..3 (cols 4..7 zero), block B experts 4..7
    # (cols 0..3 zero) — out partition index is relative to the sliced lhsT.
    w3p = np.zeros((128, 16), f4)
    for e in range(E):
        col = e if e < 4 else 8 + e
        w3p[(e % 4) * 32 : (e % 4) * 32 + 32, col] = out_w[e, 0, :]
    wr = np.ascontiguousarray(router_w.T).astype(f4)  # [64, 8]
    # bias columns
    biasp = np.zeros((128, 8), f4)
    b1col = np.concatenate(
        [l1_b[:, L2], l1_b[:, :L2].T.reshape(120)]
    )  # matches stacked f
    biasp[:, 0] = SQ_SCALE * b1col
    biasp[:, 1] = b1col
    biasp[:, 2] = l2_b[0:4].reshape(128)
    biasp[:, 3] = l2_b[4:8].reshape(128)
    biasp[0:8, 4] = l1_b[:, L2] + out_b[:, 0]
    biasp[0:8, 5] = router_b
    idn = np.eye(E, dtype=f4)
    return {"w1t": w1t, "w2p": w2p, "w3p": w3p, "wr": wr, "biasp": biasp, "idn": idn}


_cache = {}
_last_results = None


def kernel(x, router_w, router_b, l1_w, l1_b, l2_w, l2_b, out_w, out_b):
    global _last_results
    x = np.asarray(x, dtype=np.float32)
    weights = prep_weights(
        np.asarray(router_w, np.float32),
        np.asarray(router_b, np.float32),
        np.asarray(l1_w, np.float32),
        np.asarray(l1_b, np.float32),
        np.asarray(l2_w, np.float32),
        np.asarray(l2_b, np.float32),
        np.asarray(out_w, np.float32),
        np.asarray(out_b, np.float32),
    )

    xT_full = np.ascontiguousarray(x.T)  # [L1, B]
    in_maps = []
    for c in range(N_CORES):
        shard = np.ascontiguousarray(xT_full[:, c * B_SH : (c + 1) * B_SH])
        in_maps.append({"xT": shard, **weights})

    if "nc" not in _cache:
        _cache["nc"] = build_nc(mb=int(os.environ.get("KERNEL_MB", "512")))
    nc = _cache["nc"]

    from concourse.bass_utils import run_bass_kernel_spmd

    trace = bool(int(os.environ.get("KERNEL_TRACE", "0")))
    try:
        res = run_bass_kernel_spmd(
            nc, in_maps, core_ids=list(range(N_CORES)), trace=trace
        )
    except Exception:
        if not trace:
            raise
        res = run_bass_kernel_spmd(
            nc, in_maps, core_ids=list(range(N_CORES)), trace=False
        )
    _last_results = res
    out = np.concatenate(
        [np.ascontiguousarray(r["y"].T).reshape(B_SH, 1) for r in res.results], axis=0
    )
    return out



# revision 4
# speedup vs baseline: 1.8038x; 1.8038x over previous
"""Trainium2 Bass kernel for nn_MoELayerStacks (moe_routing).

Full inputs in, full output out. Data-parallel over batch across 8 cores.

Math (per batch row b):
  gate = [x[:32], x[1536:1568]] @ router_w.T + router_b           # [8]
  idx  = argmax(gate)
  l1c  = x @ l1_w[e].T + l1_b[e]   for all e                      # [8, 16]
  l1x  = clip([square(l1c[:, :15])*255/256, l1c[:, :15]], 0, 1)   # [8, 30]
  l2x  = clip(l1x @ l2_w[e].T + l2_b[e], 0, 1)                    # [8, 32]
  out  = (l2x @ out_w[e].T + out_b[e] + l1c[:, 15])[idx]          # [1]

v2 design (vs the fp32r v1): x and the expert weights are cast to fp16 on
the host, halving HBM->SBUF traffic (the dominant cost) and keeping the PE
at 1 cyc/row. The 64 router features ride in a separate fp32 sidecar so the
argmax sees the exact fp32 gate dot products (identical numerics to v1).

Layouts: features on partitions, batch on the free dim for l1/l2. Stacked
l1 feature index r(e,o): l1x features (k = o*8+e) at r = k for k < 64 and
r = k+8 for k >= 64; the 8 l1x_out features at r = 64+e so a lane-aligned
DVE copy can drop them into rows 64..71 of the fp32 gate-stationary tile.

The gate, l3, and argmax-select run batch-major without any PE transposes:
per 128-column chunk j, one fp32 matmul with stationary xr_ext[:, j] (64 xr
rows + 8 l1x_out rows + a ones row for router_b) and moving wcomb [97, 16]
produces gate (cols 0..7) and l1x_out (cols 8..15, via an identity block);
two fp16 matmuls with stationary l2a/l2b chunks and moving w3e accumulate
the l3 contribution into cols 8..15. A short DVE chain (reduce_max, is_ge,
mult, reduce_sum) then emits the selected expert output per batch row.
"""

import os
from contextlib import ExitStack

import numpy as np

import concourse.bacc as bacc
import concourse.mybir as mybir
import concourse.tile as tile

N_CORES = 8
B, L1, L2, L3, E = 16384, 3072, 15, 32, 8
RF = 32  # router feats per perspective
HALF = L1 // 2
B_SH = B // N_CORES  # 2048 rows per core
KC = L1 // 128  # 24 contraction chunks
SQ_SCALE = float(np.sqrt(255.0 / 256.0))
MB = 512  # batch columns per block
NB = B_SH // MB  # 4 blocks
NSUB = MB // 128  # 4 128-col chunks per block

F32 = mybir.dt.float32
F16 = mybir.dt.float16
ALU = mybir.AluOpType


def _stack_row(k):
    """Stacked l1 partition for l1x feature k = o*8+e (l1x_out at 64..71)."""
    return k if k < 64 else k + 8


def build_nc():
    nc = bacc.Bacc(dynamic_dma_scratch_size=2048)

    xp = nc.dram_tensor("xp", [128, NB * KC * MB], F16, kind="ExternalInput")
    xr = nc.dram_tensor("xr", [2 * RF, B_SH], F32, kind="ExternalInput")
    w1t = nc.dram_tensor("w1t", [128, KC * 128], F16, kind="ExternalInput")
    w2p = nc.dram_tensor("w2p", [128, 512], F16, kind="ExternalInput")
    w3p = nc.dram_tensor("w3p", [128, 32], F16, kind="ExternalInput")
    wc = nc.dram_tensor("wc", [128, 16], F32, kind="ExternalInput")
    biasp = nc.dram_tensor("biasp", [128, 8], F32, kind="ExternalInput")
    y = nc.dram_tensor("y", [128, NB * NSUB], F32, kind="ExternalOutput")

    with tile.TileContext(nc) as tc, ExitStack() as ctx:
        const = ctx.enter_context(tc.tile_pool(name="const", bufs=1))
        xpool = ctx.enter_context(tc.tile_pool(name="x", bufs=3))
        actp = ctx.enter_context(tc.tile_pool(name="act", bufs=2))
        smallp = ctx.enter_context(tc.tile_pool(name="small", bufs=2))
        ps_1 = ctx.enter_context(tc.tile_pool(name="ps1", bufs=3, space="PSUM"))
        ps_2a = ctx.enter_context(tc.tile_pool(name="ps2a", bufs=1, space="PSUM"))
        ps_2b = ctx.enter_context(tc.tile_pool(name="ps2b", bufs=1, space="PSUM"))
        ps_sel = ctx.enter_context(tc.tile_pool(name="psel", bufs=2, space="PSUM"))

        w1t_sb = const.tile([128, KC, 128], F16)
        w1t_v = w1t[:, :].rearrange("p (c f) -> p c f", f=128)
        w2_sb = const.tile([128, 512], F16)
        w3_sb = const.tile([128, 2, 16], F16)
        wc_sb = const.tile([128, 16], F32)
        bias_sb = const.tile([128, 8], F32)
        xre = const.tile([128, B_SH], F32)  # rows 0..63 xr, 64..71 l1x_out, 96 ones
        yfull = const.tile([128, NB * NSUB], F32)

        def emit_consts():
            nc.sync.dma_start(w1t_sb[:, 6:24, :], w1t_v[:, 6:24, :])
            nc.sync.dma_start(w2_sb[:], w2p[:, :])
            nc.sync.dma_start(
                w3_sb[:], w3p[:, :].rearrange("p (g f) -> p g f", f=16)
            )
            nc.sync.dma_start(wc_sb[:], wc[:, :])
            nc.sync.dma_start(bias_sb[:], biasp[:, :])
            nc.scalar.dma_start(xre[0 : 2 * RF, :], xr[:, :])
            nc.vector.memset(xre[64:128, :], 0.0)
            nc.vector.memset(xre[96:97, :], 1.0)

        st = {}

        def emit_load(b, pieces=4):
            xt = xpool.tile([128, KC, MB], F16, tag="xt")
            off = b * KC * MB
            per = KC // pieces
            src = xp[:, off : off + KC * MB].rearrange("p (c m) -> p c m", m=MB)
            for i in range(pieces):
                eng = nc.sync if (b + i) % 2 == 0 else nc.scalar
                eng.dma_start(
                    xt[:, i * per : (i + 1) * per, :], src[:, i * per : (i + 1) * per, :]
                )
            st[b] = {"xt": xt}

        def emit_burst(b):
            xt = st[b]["xt"]
            ps1 = ps_1.tile([128, MB], F32, tag="ps1")
            for c in range(KC):
                nc.tensor.matmul(
                    ps1[:],
                    w1t_sb[:, c, :],
                    xt[:, c, :],
                    start=(c == 0),
                    stop=(c == KC - 1),
                )
            st[b]["ps1"] = ps1

        def emit_tail(b):
            m0 = b * MB
            ps1 = st[b]["ps1"]
            # l1x_out (+ l1 bias + out_b) into the fp32 gate-stationary rows
            nc.vector.tensor_scalar(
                xre[64:72, m0 : m0 + MB],
                ps1[64:72, :],
                bias_sb[64:72, 4:5],
                None,
                op0=ALU.add,
            )
            # sq = min(1, (s*(l1c+b1))^2) ; raw = min(1, max(0, l1c+b1))
            sq = actp.tile([128, MB], F16, tag="sq")
            nc.vector.tensor_scalar(
                sq[:], ps1[:], bias_sb[:, 1:2], SQ_SCALE, op0=ALU.add, op1=ALU.mult
            )
            nc.vector.tensor_tensor(sq[:], sq[:], sq[:], op=ALU.mult)
            nc.vector.tensor_scalar_min(sq[:], sq[:], 1.0)
            raw = actp.tile([128, MB], F16, tag="raw")
            nc.vector.tensor_scalar(
                raw[:], ps1[:], bias_sb[:, 1:2], 0.0, op0=ALU.add, op1=ALU.max
            )
            nc.vector.tensor_scalar_min(raw[:], raw[:], 1.0)

            # l2: two expert groups (0-3, 4-7), sq+raw accumulated
            ps2a = ps_2a.tile([128, MB], F32, tag="ps2a")
            nc.tensor.matmul(ps2a[:], w2_sb[:, 0:128], sq[:], start=True, stop=False)
            nc.tensor.matmul(ps2a[:], w2_sb[:, 128:256], raw[:], start=False, stop=True)
            ps2b = ps_2b.tile([128, MB], F32, tag="ps2b")
            nc.tensor.matmul(ps2b[:], w2_sb[:, 256:384], sq[:], start=True, stop=False)
            nc.tensor.matmul(ps2b[:], w2_sb[:, 384:512], raw[:], start=False, stop=True)

            l2a = actp.tile([128, MB], F16, tag="l2a")
            nc.vector.tensor_scalar(
                l2a[:], ps2a[:], bias_sb[:, 2:3], 0.0, op0=ALU.add, op1=ALU.max
            )
            nc.vector.tensor_scalar_min(l2a[:], l2a[:], 1.0)
            l2b = actp.tile([128, MB], F16, tag="l2b")
            nc.vector.tensor_scalar(
                l2b[:], ps2b[:], bias_sb[:, 3:4], 0.0, op0=ALU.add, op1=ALU.max
            )
            nc.vector.tensor_scalar_min(l2b[:], l2b[:], 1.0)

            # batch-major gate + all_outputs: per 128-col chunk j, PSUM [128, 16]
            # cols 0..7 = gate (fp32, exact), cols 8..15 = l1x_out + l3c
            psel = ps_sel.tile([128, NSUB, 16], F32, tag="psel")
            for j in range(NSUB):
                c0 = m0 + j * 128
                nc.tensor.matmul(
                    psel[:, j, :],
                    xre[0:97, c0 : c0 + 128],
                    wc_sb[0:97, :],
                    start=True,
                    stop=False,
                )
                nc.tensor.matmul(
                    psel[:, j, :],
                    l2a[:, j * 128 : (j + 1) * 128],
                    w3_sb[:, 0, :],
                    start=False,
                    stop=False,
                    skip_group_check=True,
                )
                nc.tensor.matmul(
                    psel[:, j, :],
                    l2b[:, j * 128 : (j + 1) * 128],
                    w3_sb[:, 1, :],
                    start=False,
                    stop=True,
                    skip_group_check=True,
                )

            # argmax-select, batch on partitions
            mx = smallp.tile([128, NSUB], F32, tag="mx")
            nc.vector.reduce_max(mx[:], psel[:, :, 0:8], axis=mybir.AxisListType.X)
            eq = smallp.tile([128, NSUB, 8], F32, tag="eq")
            for j in range(NSUB):
                nc.vector.tensor_scalar(
                    eq[:, j, :], psel[:, j, 0:8], mx[:, j : j + 1], None, op0=ALU.is_ge
                )
            prod = smallp.tile([128, NSUB, 8], F32, tag="prod")
            nc.vector.tensor_tensor(prod[:], eq[:], psel[:, :, 8:16], op=ALU.mult)
            nc.vector.reduce_sum(
                yfull[:, b * NSUB : (b + 1) * NSUB], prod[:], axis=mybir.AxisListType.X
            )
            del st[b]

        # skewed software pipeline: PE runs burst(b) before tail(b-1) so the
        # DVE tail chain of block b-1 has a full block of slack.
        nc.sync.dma_start(w1t_sb[:, 0:6, :], w1t_v[:, 0:6, :])
        emit_load(0, pieces=8)
        emit_consts()
        emit_burst(0)
        for b in range(1, NB):
            emit_load(b)
            emit_burst(b)
            emit_tail(b - 1)
        emit_tail(NB - 1)
        nc.sync.dma_start(y[:, :], yfull[:])

    nc.finalize()
    return nc


def prep_weights(router_w, router_b, l1_w, l1_b, l2_w, l2_b, out_w, out_b):
    """Host-side packing of the (tiny) weights into the kernel's layouts."""
    f4, f2 = np.float32, np.float16
    # stacked l1 rows: l1x k=o*8+e -> r(k); l1x_out e -> 64+e
    w1_stacked = np.zeros((128, L1), f4)
    b1col = np.zeros(128, f4)
    for o in range(L2):
        for e in range(E):
            r = _stack_row(o * 8 + e)
            w1_stacked[r] = l1_w[e, o, :]
            b1col[r] = l1_b[e, o]
    for e in range(E):
        w1_stacked[64 + e] = l1_w[e, L2, :]
        b1col[64 + e] = l1_b[e, L2]
    w1t_kf = np.ascontiguousarray(w1_stacked.T).astype(f2)  # [L1, 128]
    # swizzle to [p, c, f] so the on-chip load is contiguous per partition
    w1t = np.ascontiguousarray(
        np.transpose(w1t_kf.reshape(KC, 128, 128), (1, 0, 2))
    ).reshape(128, KC * 128)
    # l2 weights: rows r(e,o), packed [sqA | rawA | sqB | rawB]
    w2p = np.zeros((128, 512), f4)
    for e in range(E):
        base = 0 if e < 4 else 256
        c0 = (e % 4) * 32
        wt = l2_w[e].T  # [30, 32]; rows 0..14 sq features, 15..29 raw
        rows = np.array([_stack_row(o * 8 + e) for o in range(L2)])
        w2p[rows, base + c0 : base + c0 + 32] = wt[0:L2]
        w2p[rows, base + 128 + c0 : base + 128 + c0 + 32] = wt[L2 : 2 * L2]
    w2p = w2p.astype(f2)
    # l3 (batch-major): w3p[:, g*16 + 8 + e] over the 32-feature band of e
    w3p = np.zeros((128, 32), f4)
    for e in range(E):
        g = e // 4
        w3p[(e % 4) * 32 : (e % 4) * 32 + 32, g * 16 + 8 + e] = out_w[e, 0, :]
    w3p = w3p.astype(f2)
    # wcomb: rows 0..63 router_w.T -> gate cols; rows 64..71 identity -> l1x_out
    # passthrough; row 96 (ones row in xre) carries router_b
    wcp = np.zeros((128, 16), f4)
    wcp[0 : 2 * RF, 0:8] = router_w.T
    for e in range(E):
        wcp[64 + e, 8 + e] = 1.0
    wcp[96, 0:8] = router_b
    biasp = np.zeros((128, 8), f4)
    biasp[:, 1] = b1col
    biasp[:, 2] = l2_b[0:4].reshape(128)
    biasp[:, 3] = l2_b[4:8].reshape(128)
    biasp[64:72, 4] = l1_b[:, L2] + out_b[:, 0]
    return {"w1t": w1t, "w2p": w2p, "w3p": w3p, "wc": wcp, "biasp": biasp}


_cache = {}
_last_results = None


def kernel(x, router_w, router_b, l1_w, l1_b, l2_w, l2_b, out_w, out_b):
    global _last_results
    x = np.asarray(x, dtype=np.float32)
    weights = prep_weights(
        np.asarray(router_w, np.float32),
        np.asarray(router_b, np.float32),
        np.asarray(l1_w, np.float32),
        np.asarray(l1_b, np.float32),
        np.asarray(l2_w, np.float32),
        np.asarray(l2_b, np.float32),
        np.asarray(out_w, np.float32),
        np.asarray(out_b, np.float32),
    )

    xh = x.astype(np.float16)
    in_maps = []
    for core in range(N_CORES):
        shard = xh[core * B_SH : (core + 1) * B_SH]  # [2048, 3072] f16
        # xp[p, b, c, m] = shard[b*MB + m, c*128 + p]
        xp = np.ascontiguousarray(
            shard.reshape(NB, MB, KC, 128).transpose(3, 0, 2, 1)
        ).reshape(128, NB * KC * MB)
        sh32 = x[core * B_SH : (core + 1) * B_SH]
        xr = np.ascontiguousarray(
            np.concatenate([sh32[:, :RF], sh32[:, HALF : HALF + RF]], axis=1).T
        )  # [64, 2048] f32
        in_maps.append({"xp": xp, "xr": xr, **weights})

    if "nc" not in _cache:
        _cache["nc"] = build_nc()
    nc = _cache["nc"]

    from concourse.bass_utils import run_bass_kernel_spmd

    trace = bool(int(os.environ.get("KERNEL_TRACE", "0")))
    try:
        res = run_bass_kernel_spmd(
            nc, in_maps, core_ids=list(range(N_CORES)), trace=trace
        )
    except Exception:
        if not trace:
            raise
        res = run_bass_kernel_spmd(
            nc, in_maps, core_ids=list(range(N_CORES)), trace=False
        )
    _last_results = res
    # y[p, g] = out row g*128 + p within the core shard
    out = np.concatenate(
        [np.ascontiguousarray(r["y"].T).reshape(B_SH, 1) for r in res.results], axis=0
    )
    return out
